# revision 1
# baseline (speedup 1.0000x reference)
"""Trainium2 Bass kernel for per-element tiny MLPs.

Problem: N=4,000,000 independent 1->8->1 MLPs:
    y[i] = W2[i] @ relu(W1[i] * x[i] + b1[i]) + b2[i]

Memory-bound: 104 B/net in + 4 B/net out. Sharded over 8 NeuronCores by net
index (data parallel, no communication).

Device layout (per core, R=500,000 nets padded to R_PAD=128*3907): natural
interleaved layout — tile t covers 128*Fi nets; partition p holds nets
[base + p*Fi, base + (p+1)*Fi); the hidden dim j stays innermost in the free
dim, i.e. a weight tile is [128, Fi*8] and is a contiguous slice of the
natural [N, 8] array. No host-side repacking beyond pad+slice, and every DMA
is a full-width 128-partition contiguous transfer.

Per tile (everything in the free dim; no PE, no PSUM):
  DVE : z = broadcast(x) * W1    (in0 carries a step-0 inner AP dim — exact,
                                  runs at the same 1x rate as a plain TT)
  DVE : z = z + b1
  ACT : h = relu(z)
  DVE : u = h * W2
  DVE : y = segmented_reduce_8(u)  (tensor_reduce axis=X on [128,Fi,8])
  DVE : y = y + b2

Engine budget per core: DVE ~140us (the wall), ACT ~30us, DMA ~135us.
GPSIMD/PE deliberately idle: GPSIMD elementwise steals DVE SBUF ports
(negative-sum), and PE fp32 matmuls run 4 cyc/row + HAM-throttled at low
duty. All ops are non-in-place: in-place DVE ops measured ~2x slower.
"""

import numpy as np
from contextlib import ExitStack

import concourse.bacc as bacc
import concourse.mybir as mybir
import concourse.tile as tile
from concourse.bass_utils import run_bass_kernel_spmd

F32 = mybir.dt.float32
AF = mybir.ActivationFunctionType
OP = mybir.AluOpType
AX = mybir.AxisListType

N = 4_000_000
H = 8
N_CORES = 8
R = N // N_CORES            # 500,000 nets per core
FP = 3907                   # nets per partition (padded): 128*3907 = 500,096
R_PAD = 128 * FP
FIS = [32, 256] + [288] * 12 + [163]   # small first tile primes the pipeline


def build_nc(fis):
    fp = sum(fis)
    rp = 128 * fp

    nc = bacc.Bacc("TRN2", target_bir_lowering=False, debug=False)

    w1 = nc.dram_tensor("w1", [rp, H], F32, kind="ExternalInput")
    b1 = nc.dram_tensor("b1", [rp, H], F32, kind="ExternalInput")
    w2 = nc.dram_tensor("w2", [rp, H], F32, kind="ExternalInput")
    xs = nc.dram_tensor("xs", [rp], F32, kind="ExternalInput")
    b2 = nc.dram_tensor("b2", [rp], F32, kind="ExternalInput")
    ys = nc.dram_tensor("ys", [rp], F32, kind="ExternalOutput")

    with tile.TileContext(nc) as tc, ExitStack() as ctx:
        wpool = ctx.enter_context(tc.tile_pool(name="w", bufs=3))
        zpool = ctx.enter_context(tc.tile_pool(name="z", bufs=2))
        vpool = ctx.enter_context(tc.tile_pool(name="v", bufs=4))

        nb = 0
        for fi in fis:
            nrows = 128 * fi
            wsl = lambda t: t.ap()[nb:nb + nrows, :].rearrange(
                "(p f) j -> p (f j)", p=128
            )
            vsl = lambda t: t.ap()[nb:nb + nrows].rearrange("(p f) -> p f", p=128)

            xt = vpool.tile([128, fi], F32, tag="xt")
            nc.scalar.dma_start(xt[:], vsl(xs))
            w1t = wpool.tile([128, fi * H], F32, tag="w1t", bufs=4)
            nc.sync.dma_start(w1t[:], wsl(w1))
            b1t = wpool.tile([128, fi * H], F32, tag="b1t")
            nc.scalar.dma_start(b1t[:], wsl(b1))
            w2t = wpool.tile([128, fi * H], F32, tag="w2t")
            nc.sync.dma_start(w2t[:], wsl(w2))
            b2t = vpool.tile([128, fi], F32, tag="b2t")
            nc.scalar.dma_start(b2t[:], vsl(b2))

            w1t3 = w1t[:].rearrange("p (f j) -> p f j", j=H)
            xb = xt[:].broadcast_to([128, fi, H])

            za = zpool.tile([128, fi * H], F32, tag="za")
            zb = zpool.tile([128, fi * H], F32, tag="zb")
            zc = zpool.tile([128, fi * H], F32, tag="zc")
            zd = zpool.tile([128, fi * H], F32, tag="zd")

            nc.vector.tensor_tensor(
                za[:].rearrange("p (f j) -> p f j", j=H), xb, w1t3, op=OP.mult
            )
            nc.vector.tensor_tensor(zb[:], za[:], b1t[:], op=OP.add)
            nc.scalar.activation(zc[:], zb[:], AF.Relu)
            nc.vector.tensor_tensor(zd[:], zc[:], w2t[:], op=OP.mult)

            yt = vpool.tile([128, fi], F32, tag="yt")
            nc.vector.tensor_reduce(
                yt[:], zd[:].rearrange("p (f j) -> p f j", j=H), axis=AX.X, op=OP.add
            )
            yo = vpool.tile([128, fi], F32, tag="yo")
            nc.vector.tensor_tensor(yo[:], yt[:], b2t[:], op=OP.add)

            nc.scalar.dma_start(vsl(ys), yo[:])
            nb += nrows

    nc.compile()
    return nc


# ---------------- entry point ----------------

_CACHE = {}


def _get_nc():
    if "nc" not in _CACHE:
        _CACHE["nc"] = build_nc(FIS)
    return _CACHE["nc"]


def _pad2(a):
    out = np.zeros((R_PAD, H), np.float32)
    out[:R] = a
    return out


def _pad1(a):
    out = np.zeros(R_PAD, np.float32)
    out[:R] = a
    return out


def _make_in_maps(x, W1, b1, W2, b2):
    x = np.ascontiguousarray(np.asarray(x), np.float32)
    W1 = np.ascontiguousarray(np.asarray(W1), np.float32)
    b1 = np.ascontiguousarray(np.asarray(b1), np.float32)
    W2 = np.ascontiguousarray(np.asarray(W2), np.float32)
    b2 = np.ascontiguousarray(np.asarray(b2), np.float32)
    in_maps = []
    for c in range(N_CORES):
        sl = slice(c * R, (c + 1) * R)
        in_maps.append({
            "w1": _pad2(W1[sl]),
            "b1": _pad2(b1[sl]),
            "w2": _pad2(W2[sl]),
            "xs": _pad1(x[sl, 0]),
            "b2": _pad1(b2[sl, 0]),
        })
    return in_maps


def _run(x, W1, b1, W2, b2, **kw):
    nc = _get_nc()
    res = run_bass_kernel_spmd(nc, _make_in_maps(x, W1, b1, W2, b2),
                               core_ids=list(range(N_CORES)), **kw)
    y = np.empty((N, 1), np.float32)
    for c in range(N_CORES):
        y[c * R:(c + 1) * R, 0] = res.results[c]["ys"].reshape(-1)[:R]
    return y, res


def kernel(x, W1, b1, W2, b2):
    y, _ = _run(x, W1, b1, W2, b2)
    return y



# revision 3
# speedup vs baseline: 1.3053x; 1.3053x over previous
"""Trainium2 Bass kernel for per-element tiny MLPs (fp16 rewrite).

Problem: N=4,000,000 independent 1->8->1 MLPs:
    y[i] = W2[i] @ relu(W1[i] * x[i] + b1[i]) + b2[i]

Memory-bound + DVE-bound. Sharded over 8 NeuronCores by net index (data
parallel, no communication).

Key changes vs the fp32 baseline (179-215us):
  * fp16 everywhere: halves HBM traffic (52B/net in, 2B out) and gives
    tensor_tensor the 2x_1p DVE perf mode (2 elem/cyc/lane vs 1).
    Measured host-side accuracy: rel_l2 ~ 5e-4 (budget 2e-2).
  * hidden-dim-OUTER device layout: a weight tile is [128, 8*f] with the
    hidden index j as the outer free-dim block, so the per-net segmented
    sum becomes a 3-step tree of CONTIGUOUS tensor_tensor adds at 2x mode
    (tensor_reduce has no accelerated mode and cost 8f cycles at 1x; the
    tree costs 3.5f at 2x + the +b2 op).
  * fused input streams: one [128, 24f] DMA per tile carries w1|b1|w2
    (j-outer, host-packed) and one [128, 2f] DMA carries x|b2 - 3 DMA
    dispatches per tile instead of 6, all >=512B/partition contiguous.
  * relu stays on the otherwise-idle ACT engine (scalar_tensor_tensor
    fusion would drop the op count but runs at 1x: net loss).

Per-core budget (500,224 padded nets): DVE 7 ops/tile, sum(7*58 + 16*f)
~ 66K cyc ~ 69us busy; DMA 27MB at ~420GB/s ~ 64us; ACT relu ~ 27us.
"""

import numpy as np
from contextlib import ExitStack

import concourse.bacc as bacc
import concourse.mybir as mybir
import concourse.tile as tile
from concourse.bass_utils import run_bass_kernel_spmd

F16 = mybir.dt.float16
AF = mybir.ActivationFunctionType
OP = mybir.AluOpType

N = 4_000_000
H = 8
N_CORES = 8
R = N // N_CORES            # 500,000 nets per core
FP = 3908                   # free-dim cols per partition: 128*3908 = 500,224
R_PAD = 128 * FP
# Ramp-up tile sizes (sum = FP): small first tiles get DVE started while
# the big steady-state DMAs stream in.
FIS = [32, 96, 256, 512, 640, 640, 640, 640, 452]
assert sum(FIS) == FP and all(f % 2 == 0 for f in FIS)


def build_nc(fis):
    fp = sum(fis)

    nc = bacc.Bacc("TRN2", target_bir_lowering=False, debug=False)

    pk = nc.dram_tensor("pk", [128, 24 * fp], F16, kind="ExternalInput")
    sm = nc.dram_tensor("sm", [128, 2 * fp], F16, kind="ExternalInput")
    ys = nc.dram_tensor("ys", [128, fp], F16, kind="ExternalOutput")

    with tile.TileContext(nc) as tc, ExitStack() as ctx:
        wpool = ctx.enter_context(tc.tile_pool(name="w", bufs=2))
        zpool = ctx.enter_context(tc.tile_pool(name="z", bufs=2))
        vpool = ctx.enter_context(tc.tile_pool(name="v", bufs=2))

        nbf = 0
        for fi in fis:
            wt = wpool.tile([128, 24 * fi], F16, tag="wt")
            nc.sync.dma_start(wt[:], pk.ap()[:, 24 * nbf:24 * (nbf + fi)])
            st = vpool.tile([128, 2 * fi], F16, tag="st")
            nc.sync.dma_start(st[:], sm.ap()[:, 2 * nbf:2 * (nbf + fi)])

            w1 = wt[:][:, 0:8 * fi].rearrange("p (j f) -> p j f", j=H)
            b1 = wt[:][:, 8 * fi:16 * fi]
            w2 = wt[:][:, 16 * fi:24 * fi]
            xa = st[:][:, 0:fi]
            b2a = st[:][:, fi:2 * fi]
            xb = xa.rearrange("p f -> p () f").broadcast_to([128, H, fi])

            za = zpool.tile([128, 8 * fi], F16, tag="za")
            nc.vector.tensor_tensor(
                za[:].rearrange("p (j f) -> p j f", j=H), xb, w1, op=OP.mult
            )
            zb = zpool.tile([128, 8 * fi], F16, tag="zb")
            nc.vector.tensor_tensor(zb[:], za[:], b1, op=OP.add)
            hc = zpool.tile([128, 8 * fi], F16, tag="hc")
            nc.scalar.activation(hc[:], zb[:], AF.Relu)
            u = zpool.tile([128, 8 * fi], F16, tag="u")
            nc.vector.tensor_tensor(u[:], hc[:], w2, op=OP.mult)

            r1 = vpool.tile([128, 4 * fi], F16, tag="r1")
            nc.vector.tensor_tensor(r1[:], u[:][:, 0:4 * fi], u[:][:, 4 * fi:8 * fi],
                                    op=OP.add)
            r2 = vpool.tile([128, 2 * fi], F16, tag="r2")
            nc.vector.tensor_tensor(r2[:], r1[:][:, 0:2 * fi], r1[:][:, 2 * fi:4 * fi],
                                    op=OP.add)
            r3 = vpool.tile([128, fi], F16, tag="r3")
            nc.vector.tensor_tensor(r3[:], r2[:][:, 0:fi], r2[:][:, fi:2 * fi],
                                    op=OP.add)
            yo = vpool.tile([128, fi], F16, tag="yo")
            nc.vector.tensor_tensor(yo[:], r3[:], b2a, op=OP.add)

            nc.scalar.dma_start(ys.ap()[:, nbf:nbf + fi], yo[:])
            nbf += fi

    nc.compile()
    return nc


# ---------------- host-side pack / unpack ----------------

_CACHE = {}


def _get_nc():
    if "nc" not in _CACHE:
        _CACHE["nc"] = build_nc(FIS)
    return _CACHE["nc"]


def _make_in_maps(x, W1, b1, W2, b2):
    x = np.asarray(x, np.float32).reshape(N)
    W1 = np.asarray(W1, np.float32)
    b1 = np.asarray(b1, np.float32)
    W2 = np.asarray(W2, np.float32)
    b2 = np.asarray(b2, np.float32).reshape(N)

    in_maps = []
    for c in range(N_CORES):
        sl = slice(c * R, (c + 1) * R)
        w1p = np.zeros((R_PAD, H), np.float16); w1p[:R] = W1[sl]
        b1p = np.zeros((R_PAD, H), np.float16); b1p[:R] = b1[sl]
        w2p = np.zeros((R_PAD, H), np.float16); w2p[:R] = W2[sl]
        xp = np.zeros(R_PAD, np.float16); xp[:R] = x[sl]
        b2p = np.zeros(R_PAD, np.float16); b2p[:R] = b2[sl]

        pk = np.empty((128, 24 * FP), np.float16)
        sm = np.empty((128, 2 * FP), np.float16)
        nbf = 0
        for fi in FIS:
            rows = slice(128 * nbf, 128 * (nbf + fi))
            # [128*fi, 8] -> [128, fi, 8] -> j-outer [128, 8, fi] -> flat
            pk[:, 24 * nbf:24 * nbf + 8 * fi] = \
                w1p[rows].reshape(128, fi, H).transpose(0, 2, 1).reshape(128, 8 * fi)
            pk[:, 24 * nbf + 8 * fi:24 * nbf + 16 * fi] = \
                b1p[rows].reshape(128, fi, H).transpose(0, 2, 1).reshape(128, 8 * fi)
            pk[:, 24 * nbf + 16 * fi:24 * (nbf + fi)] = \
                w2p[rows].reshape(128, fi, H).transpose(0, 2, 1).reshape(128, 8 * fi)
            sm[:, 2 * nbf:2 * nbf + fi] = xp[rows].reshape(128, fi)
            sm[:, 2 * nbf + fi:2 * (nbf + fi)] = b2p[rows].reshape(128, fi)
            nbf += fi
        in_maps.append({"pk": pk, "sm": sm})
    return in_maps


def _unpack_out(res):
    y = np.empty((N, 1), np.float32)
    for c in range(N_CORES):
        ysc = res.results[c]["ys"].reshape(128, FP)
        yflat = np.empty(R_PAD, np.float32)
        nbf = 0
        for fi in FIS:
            yflat[128 * nbf:128 * (nbf + fi)] = \
                ysc[:, nbf:nbf + fi].astype(np.float32).reshape(-1)
            nbf += fi
        y[c * R:(c + 1) * R, 0] = yflat[:R]
    return y


def _run(x, W1, b1, W2, b2, **kw):
    nc = _get_nc()
    res = run_bass_kernel_spmd(nc, _make_in_maps(x, W1, b1, W2, b2),
                               core_ids=list(range(N_CORES)), **kw)
    return _unpack_out(res), res


def kernel(x, W1, b1, W2, b2):
    y, _ = _run(x, W1, b1, W2, b2)
    return y


# revision 4
# speedup vs baseline: 1.6626x; 1.2737x over previous
"""Trainium2 Bass kernel for per-element tiny MLPs (fp16, software-pipelined).

Problem: N=4,000,000 independent 1->8->1 MLPs:
    y[i] = W2[i] @ relu(W1[i] * x[i] + b1[i]) + b2[i]

Memory-bound + DVE-bound. Sharded over 8 NeuronCores by net index (data
parallel, no communication).

Design (vs the 179-215us fp32 baseline):
  * fp16 everywhere: halves HBM traffic (52B/net in, 2B out) and gives
    tensor_tensor the 2x_1p DVE perf mode (0.52ns/elem measured, vs
    1.04 for fp32). Host-side accuracy sim: rel_l2 ~ 5e-4 (budget 2e-2).
  * hidden-dim-OUTER device layout: a weight tile is [128, 8*f] with the
    hidden index j as the outer free-dim block, so the per-net segmented
    sum is a 3-step tree of CONTIGUOUS 2x-mode tensor_tensor adds
    (tensor_reduce has no fast mode: 1 elem/cyc).
  * software pipeline: per tile, phase A = {mult x*W1, add b1} and
    phase B = {mult *W2, 3-level tree, +b2}; emitted as A_t, B_{t-1} so
    the in-order DVE stream always has B-work of the previous tile while
    ACT runs relu_t. (Without this, DVE idled ~4.5us/tile waiting on
    relu: measured 137us wall.)
  * input streams split by consumer phase: one [128, 32f] DMA carries
    w1|b1 (phase A, bufs=3 for ~2 tiles of DMA lookahead), one [128,16f]
    carries w2 (phase B), one [128, 4f] carries x|b2.
  * relu stays on the otherwise-idle ACT engine; scalar_tensor_tensor
    would fuse relu+mult but runs at 1x (no fast uop): net loss.

Per-core budget: DVE ~75us busy (32 fp16 elem/net at 2x + ~165ns/op
x 56 ops, x ~1.15 DMA-contention), DMA 27MB at ~420GB/s peak ~ 64us,
ACT relu ~27us. GPSIMD/PE idle (gpsimd steals DVE SBUF ports; PE fp32
4cyc/row and PSUM results cost 1x-mode DVE post-ops).
"""

import numpy as np
from contextlib import ExitStack

import concourse.bacc as bacc
import concourse.mybir as mybir
import concourse.tile as tile
from concourse.bass_utils import run_bass_kernel_spmd

F16 = mybir.dt.float16
AF = mybir.ActivationFunctionType
OP = mybir.AluOpType

N = 4_000_000
H = 8
N_CORES = 8
R = N // N_CORES            # 500,000 nets per core
FP = 3908                   # free-dim cols per partition: 128*3908 = 500,224
R_PAD = 128 * FP
# Ramp-up then steady-state tile sizes (sum = FP, all even).
FIS = [64, 192, 452, 640, 640, 640, 640, 640]
assert sum(FIS) == FP and all(f % 2 == 0 for f in FIS)


def build_nc(fis):
    fp = sum(fis)

    nc = bacc.Bacc("TRN2", target_bir_lowering=False, debug=False)

    wa = nc.dram_tensor("wa", [128, 16 * fp], F16, kind="ExternalInput")  # w1|b1
    wb = nc.dram_tensor("wb", [128, 8 * fp], F16, kind="ExternalInput")   # w2
    sm = nc.dram_tensor("sm", [128, 2 * fp], F16, kind="ExternalInput")   # x|b2
    ys = nc.dram_tensor("ys", [128, fp], F16, kind="ExternalOutput")

    with tile.TileContext(nc) as tc, ExitStack() as ctx:
        wpool = ctx.enter_context(tc.tile_pool(name="w", bufs=2))
        zpool = ctx.enter_context(tc.tile_pool(name="z", bufs=2))
        vpool = ctx.enter_context(tc.tile_pool(name="v", bufs=2))

        state = []   # (fi, nbf, w2sl, st, hc) awaiting phase B

        def phase_a(fi, nbf):
            w12 = wpool.tile([128, 16 * fi], F16, tag="w12", bufs=3)
            nc.sync.dma_start(w12[:], wa.ap()[:, 16 * nbf:16 * (nbf + fi)])
            w2t = wpool.tile([128, 8 * fi], F16, tag="w2t")
            nc.sync.dma_start(w2t[:], wb.ap()[:, 8 * nbf:8 * (nbf + fi)])
            st = vpool.tile([128, 2 * fi], F16, tag="st", bufs=3)
            nc.sync.dma_start(st[:], sm.ap()[:, 2 * nbf:2 * (nbf + fi)])

            w1 = w12[:][:, 0:8 * fi].rearrange("p (j f) -> p j f", j=H)
            b1 = w12[:][:, 8 * fi:16 * fi]
            xb = st[:][:, 0:fi].rearrange("p f -> p () f").broadcast_to([128, H, fi])

            za = zpool.tile([128, 8 * fi], F16, tag="za", bufs=3)
            nc.vector.tensor_tensor(
                za[:].rearrange("p (j f) -> p j f", j=H), xb, w1, op=OP.mult
            )
            zb = zpool.tile([128, 8 * fi], F16, tag="zb")
            nc.vector.tensor_tensor(zb[:], za[:], b1, op=OP.add)
            hc = zpool.tile([128, 8 * fi], F16, tag="za", bufs=3)  # reuse ring
            nc.scalar.activation(hc[:], zb[:], AF.Relu)
            state.append((fi, nbf, w2t, st, hc))

        def phase_b():
            fi, nbf, w2t, st, hc = state.pop(0)
            b2a = st[:][:, fi:2 * fi]
            u = zpool.tile([128, 8 * fi], F16, tag="u")
            nc.vector.tensor_tensor(u[:], hc[:], w2t[:], op=OP.mult)
            r1 = vpool.tile([128, 4 * fi], F16, tag="r1")
            nc.vector.tensor_tensor(r1[:], u[:][:, 0:4 * fi],
                                    u[:][:, 4 * fi:8 * fi], op=OP.add)
            r2 = vpool.tile([128, 2 * fi], F16, tag="r2")
            nc.vector.tensor_tensor(r2[:], r1[:][:, 0:2 * fi],
                                    r1[:][:, 2 * fi:4 * fi], op=OP.add)
            r3 = vpool.tile([128, fi], F16, tag="r3")
            nc.vector.tensor_tensor(r3[:], r2[:][:, 0:fi], r2[:][:, fi:2 * fi],
                                    op=OP.add)
            yo = vpool.tile([128, fi], F16, tag="yo")
            nc.vector.tensor_tensor(yo[:], r3[:], b2a, op=OP.add)
            nc.scalar.dma_start(ys.ap()[:, nbf:nbf + fi], yo[:])

        nbf = 0
        for fi in fis:
            phase_a(fi, nbf)
            nbf += fi
            if len(state) > 1:
                phase_b()
        while state:
            phase_b()

    nc.compile()
    return nc


# ---------------- host-side pack / unpack ----------------

_CACHE = {}


def _get_nc():
    if "nc" not in _CACHE:
        _CACHE["nc"] = build_nc(FIS)
    return _CACHE["nc"]


def _make_in_maps(x, W1, b1, W2, b2):
    x = np.asarray(x, np.float32).reshape(N)
    W1 = np.asarray(W1, np.float32)
    b1 = np.asarray(b1, np.float32)
    W2 = np.asarray(W2, np.float32)
    b2 = np.asarray(b2, np.float32).reshape(N)

    in_maps = []
    for c in range(N_CORES):
        sl = slice(c * R, (c + 1) * R)
        w1p = np.zeros((R_PAD, H), np.float16); w1p[:R] = W1[sl]
        b1p = np.zeros((R_PAD, H), np.float16); b1p[:R] = b1[sl]
        w2p = np.zeros((R_PAD, H), np.float16); w2p[:R] = W2[sl]
        xp = np.zeros(R_PAD, np.float16); xp[:R] = x[sl]
        b2p = np.zeros(R_PAD, np.float16); b2p[:R] = b2[sl]

        wa = np.empty((128, 16 * FP), np.float16)
        wb = np.empty((128, 8 * FP), np.float16)
        sm = np.empty((128, 2 * FP), np.float16)
        nbf = 0
        for fi in FIS:
            rows = slice(128 * nbf, 128 * (nbf + fi))
            # [128*fi, 8] -> [128, fi, 8] -> j-outer [128, 8, fi] -> flat
            wa[:, 16 * nbf:16 * nbf + 8 * fi] = \
                w1p[rows].reshape(128, fi, H).transpose(0, 2, 1).reshape(128, 8 * fi)
            wa[:, 16 * nbf + 8 * fi:16 * (nbf + fi)] = \
                b1p[rows].reshape(128, fi, H).transpose(0, 2, 1).reshape(128, 8 * fi)
            wb[:, 8 * nbf:8 * (nbf + fi)] = \
                w2p[rows].reshape(128, fi, H).transpose(0, 2, 1).reshape(128, 8 * fi)
            sm[:, 2 * nbf:2 * nbf + fi] = xp[rows].reshape(128, fi)
            sm[:, 2 * nbf + fi:2 * (nbf + fi)] = b2p[rows].reshape(128, fi)
            nbf += fi
        in_maps.append({"wa": wa, "wb": wb, "sm": sm})
    return in_maps


def _unpack_out(res):
    y = np.empty((N, 1), np.float32)
    for c in range(N_CORES):
        ysc = res.results[c]["ys"].reshape(128, FP)
        yflat = np.empty(R_PAD, np.float32)
        nbf = 0
        for fi in FIS:
            yflat[128 * nbf:128 * (nbf + fi)] = \
                ysc[:, nbf:nbf + fi].astype(np.float32).reshape(-1)
            nbf += fi
        y[c * R:(c + 1) * R, 0] = yflat[:R]
    return y


def _run(x, W1, b1, W2, b2, **kw):
    nc = _get_nc()
    res = run_bass_kernel_spmd(nc, _make_in_maps(x, W1, b1, W2, b2),
                               core_ids=list(range(N_CORES)), **kw)
    return _unpack_out(res), res


def kernel(x, W1, b1, W2, b2):
    y, _ = _run(x, W1, b1, W2, b2)
    return y


# revision 6
# speedup vs baseline: 1.7079x; 1.0273x over previous
"""Trainium2 Bass kernel for per-element tiny MLPs (fp16, software-pipelined).

Problem: N=4,000,000 independent 1->8->1 MLPs:
    y[i] = W2[i] @ relu(W1[i] * x[i] + b1[i]) + b2[i]

Memory-bound + DVE-bound. Sharded over 8 NeuronCores by net index (data
parallel, no communication).

Design (vs the 179-215us fp32 baseline):
  * fp16 everywhere: halves HBM traffic (52B/net in, 2B out) and gives
    tensor_tensor the 2x_1p DVE perf mode (0.52ns/elem measured, vs
    1.04 for fp32). Host-side accuracy sim: rel_l2 ~ 5e-4 (budget 2e-2).
  * hidden-dim-OUTER device layout: a weight tile is [128, 8*f] with the
    hidden index j as the outer free-dim block, so the per-net segmented
    sum is a 3-step tree of CONTIGUOUS 2x-mode tensor_tensor adds
    (tensor_reduce has no fast mode: 1 elem/cyc).
  * software pipeline: per tile, phase A = {mult x*W1, add b1} and
    phase B = {mult *W2, 3-level tree, +b2}; emitted as A_t, B_{t-1} so
    the in-order DVE stream always has B-work of the previous tile while
    ACT runs relu_t. (Without this, DVE idled ~4.5us/tile waiting on
    relu: measured 137us wall.)
  * input streams split by consumer phase: one [128, 32f] DMA carries
    w1|b1 (phase A, bufs=3 for ~2 tiles of DMA lookahead), one [128,16f]
    carries w2 (phase B), one [128, 4f] carries x|b2.
  * relu stays on the otherwise-idle ACT engine; scalar_tensor_tensor
    would fuse relu+mult but runs at 1x (no fast uop): net loss.

Per-core budget: DVE ~75us busy (32 fp16 elem/net at 2x + ~165ns/op
x 56 ops, x ~1.15 DMA-contention), DMA 27MB at ~420GB/s peak ~ 64us,
ACT relu ~27us. GPSIMD/PE idle (gpsimd steals DVE SBUF ports; PE fp32
4cyc/row and PSUM results cost 1x-mode DVE post-ops).
"""

import numpy as np
from contextlib import ExitStack

import concourse.bacc as bacc
import concourse.mybir as mybir
import concourse.tile as tile
from concourse.bass_utils import run_bass_kernel_spmd

F16 = mybir.dt.float16
AF = mybir.ActivationFunctionType
OP = mybir.AluOpType

N = 4_000_000
H = 8
N_CORES = 8
R = N // N_CORES            # 500,000 nets per core
FP = 3908                   # free-dim cols per partition: 128*3908 = 500,224
R_PAD = 128 * FP
# Ramp-up, steady-state, ramp-down tile sizes (sum = FP, all even).
FIS = [32, 64, 128, 256, 512, 640, 640, 640, 640, 356]
assert sum(FIS) == FP and all(f % 2 == 0 for f in FIS)


def build_nc(fis):
    fp = sum(fis)

    nc = bacc.Bacc("TRN2", target_bir_lowering=False, debug=False)

    wa = nc.dram_tensor("wa", [128, 16 * fp], F16, kind="ExternalInput")  # w1|b1
    wb = nc.dram_tensor("wb", [128, 8 * fp], F16, kind="ExternalInput")   # w2
    sm = nc.dram_tensor("sm", [128, 2 * fp], F16, kind="ExternalInput")   # x|b2
    ys = nc.dram_tensor("ys", [128, fp], F16, kind="ExternalOutput")

    with tile.TileContext(nc) as tc, ExitStack() as ctx:
        wpool = ctx.enter_context(tc.tile_pool(name="w", bufs=2))
        zpool = ctx.enter_context(tc.tile_pool(name="z", bufs=2))
        vpool = ctx.enter_context(tc.tile_pool(name="v", bufs=2))

        state = []   # (fi, nbf, w2sl, st, hc) awaiting phase B

        def phase_a(fi, nbf):
            w12 = wpool.tile([128, 16 * fi], F16, tag="w12", bufs=3)
            nc.sync.dma_start(w12[:], wa.ap()[:, 16 * nbf:16 * (nbf + fi)])
            w2t = wpool.tile([128, 8 * fi], F16, tag="w2t")
            nc.sync.dma_start(w2t[:], wb.ap()[:, 8 * nbf:8 * (nbf + fi)])
            st = vpool.tile([128, 2 * fi], F16, tag="st", bufs=3)
            nc.sync.dma_start(st[:], sm.ap()[:, 2 * nbf:2 * (nbf + fi)])

            w1 = w12[:][:, 0:8 * fi].rearrange("p (j f) -> p j f", j=H)
            b1 = w12[:][:, 8 * fi:16 * fi]
            xb = st[:][:, 0:fi].rearrange("p f -> p () f").broadcast_to([128, H, fi])

            za = zpool.tile([128, 8 * fi], F16, tag="za", bufs=3)
            nc.vector.tensor_tensor(
                za[:].rearrange("p (j f) -> p j f", j=H), xb, w1, op=OP.mult
            )
            # lo/hi halves (hidden j 0..3 / 4..7): fine-grained deps so the
            # relu halves on ACT overlap the DVE stream with <=half-relu
            # exposure regardless of scheduler order.
            zb = zpool.tile([128, 8 * fi], F16, tag="zb")
            nc.vector.tensor_tensor(zb[:][:, 0:4 * fi], za[:][:, 0:4 * fi],
                                    b1[:, 0:4 * fi], op=OP.add)
            nc.vector.tensor_tensor(zb[:][:, 4 * fi:8 * fi], za[:][:, 4 * fi:8 * fi],
                                    b1[:, 4 * fi:8 * fi], op=OP.add)
            hc = zpool.tile([128, 8 * fi], F16, tag="za", bufs=3)  # reuse ring
            nc.scalar.activation(hc[:][:, 0:4 * fi], zb[:][:, 0:4 * fi], AF.Relu)
            nc.scalar.activation(hc[:][:, 4 * fi:8 * fi], zb[:][:, 4 * fi:8 * fi],
                                 AF.Relu)
            state.append((fi, nbf, w2t, st, hc))

        def phase_b():
            fi, nbf, w2t, st, hc = state.pop(0)
            b2a = st[:][:, fi:2 * fi]
            u = zpool.tile([128, 8 * fi], F16, tag="u")
            nc.vector.tensor_tensor(u[:][:, 0:4 * fi], hc[:][:, 0:4 * fi],
                                    w2t[:][:, 0:4 * fi], op=OP.mult)
            nc.vector.tensor_tensor(u[:][:, 4 * fi:8 * fi], hc[:][:, 4 * fi:8 * fi],
                                    w2t[:][:, 4 * fi:8 * fi], op=OP.mult)
            r1 = vpool.tile([128, 4 * fi], F16, tag="r1")
            nc.vector.tensor_tensor(r1[:], u[:][:, 0:4 * fi],
                                    u[:][:, 4 * fi:8 * fi], op=OP.add)
            r2 = vpool.tile([128, 2 * fi], F16, tag="r2")
            nc.vector.tensor_tensor(r2[:], r1[:][:, 0:2 * fi],
                                    r1[:][:, 2 * fi:4 * fi], op=OP.add)
            r3 = vpool.tile([128, fi], F16, tag="r3")
            nc.vector.tensor_tensor(r3[:], r2[:][:, 0:fi], r2[:][:, fi:2 * fi],
                                    op=OP.add)
            yo = vpool.tile([128, fi], F16, tag="yo")
            nc.vector.tensor_tensor(yo[:], r3[:], b2a, op=OP.add)
            nc.scalar.dma_start(ys.ap()[:, nbf:nbf + fi], yo[:])

        nbf = 0
        for fi in fis:
            phase_a(fi, nbf)
            nbf += fi
            if len(state) > 1:
                phase_b()
        while state:
            phase_b()

    nc.compile()
    return nc


# ---------------- host-side pack / unpack ----------------

_CACHE = {}


def _get_nc():
    if "nc" not in _CACHE:
        _CACHE["nc"] = build_nc(FIS)
    return _CACHE["nc"]


def _make_in_maps(x, W1, b1, W2, b2):
    x = np.asarray(x, np.float32).reshape(N)
    W1 = np.asarray(W1, np.float32)
    b1 = np.asarray(b1, np.float32)
    W2 = np.asarray(W2, np.float32)
    b2 = np.asarray(b2, np.float32).reshape(N)

    in_maps = []
    for c in range(N_CORES):
        sl = slice(c * R, (c + 1) * R)
        w1p = np.zeros((R_PAD, H), np.float16); w1p[:R] = W1[sl]
        b1p = np.zeros((R_PAD, H), np.float16); b1p[:R] = b1[sl]
        w2p = np.zeros((R_PAD, H), np.float16); w2p[:R] = W2[sl]
        xp = np.zeros(R_PAD, np.float16); xp[:R] = x[sl]
        b2p = np.zeros(R_PAD, np.float16); b2p[:R] = b2[sl]

        wa = np.empty((128, 16 * FP), np.float16)
        wb = np.empty((128, 8 * FP), np.float16)
        sm = np.empty((128, 2 * FP), np.float16)
        nbf = 0
        for fi in FIS:
            rows = slice(128 * nbf, 128 * (nbf + fi))
            # [128*fi, 8] -> [128, fi, 8] -> j-outer [128, 8, fi] -> flat
            wa[:, 16 * nbf:16 * nbf + 8 * fi] = \
                w1p[rows].reshape(128, fi, H).transpose(0, 2, 1).reshape(128, 8 * fi)
            wa[:, 16 * nbf + 8 * fi:16 * (nbf + fi)] = \
                b1p[rows].reshape(128, fi, H).transpose(0, 2, 1).reshape(128, 8 * fi)
            wb[:, 8 * nbf:8 * (nbf + fi)] = \
                w2p[rows].reshape(128, fi, H).transpose(0, 2, 1).reshape(128, 8 * fi)
            sm[:, 2 * nbf:2 * nbf + fi] = xp[rows].reshape(128, fi)
            sm[:, 2 * nbf + fi:2 * (nbf + fi)] = b2p[rows].reshape(128, fi)
            nbf += fi
        in_maps.append({"wa": wa, "wb": wb, "sm": sm})
    return in_maps


def _unpack_out(res):
    y = np.empty((N, 1), np.float32)
    for c in range(N_CORES):
        ysc = res.results[c]["ys"].reshape(128, FP)
        yflat = np.empty(R_PAD, np.float32)
        nbf = 0
        for fi in FIS:
            yflat[128 * nbf:128 * (nbf + fi)] = \
                ysc[:, nbf:nbf + fi].astype(np.float32).reshape(-1)
            nbf += fi
        y[c * R:(c + 1) * R, 0] = yflat[:R]
    return y


def _run(x, W1, b1, W2, b2, **kw):
    nc = _get_nc()
    res = run_bass_kernel_spmd(nc, _make_in_maps(x, W1, b1, W2, b2),
                               core_ids=list(range(N_CORES)), **kw)
    return _unpack_out(res), res


def kernel(x, W1, b1, W2, b2):
    y, _ = _run(x, W1, b1, W2, b2)
    return y
